# revision 42
# baseline (speedup 1.0000x reference)
"""Trainium2 Bass kernel for nn_MemNet (memory-network attention block).

Computation (per row r of B*R=5120 rows):
    fused  = tanh(cat(img, ques) @ W_fuse.T + b_fuse)          [5120, 512]
    s_j    = sum_d hist[r,j,d] * fused[r,d] * w_att[d] + b_att [5120, 10]
    attn   = softmax(s, axis=1)
    he     = sum_j attn[r,j] * hist[r,j,:]                     [5120, 512]
    he     = tanh(he @ W_hist.T + b_hist)
    out    = fused + he   -> reshape [512, 10, 512]

Pure data parallel over the leading 5120 rows -> 640 rows/core on 8 cores,
5 row-tiles of 128 rows each.  Weights replicated; activations for the big
matmul are pre-transposed on the host so the contraction dim lands on SBUF
partitions.

Final design (86772 ns best / ~87-92 ns run-to-run, vs 93423 ns session
start; all HW-measured):
  - weighted sum split 6 ACT scaled copies + 4 DVE tensor_scalar; the
    eye-matmul PSUM burst consumes tmp slices in REVERSED order so it
    launches back-to-back at the warm matmul rate instead of dripping
    behind each wsum write at the cold 630ns p-state.
  - priority knobs (HP/HP_T) measured strictly WORSE (100-114us): the
    list-scheduler's natural readiness order beats manual promotion of
    tanh1 / stage_b; leave both 0.
Earlier design notes:
  - Phased emission: all 100 mm1 matmuls (5 tiles x 20 chunks) first, then
    per-tile attention middles.  PE sustains ~427ns cadence per 512-col
    bf16 matmul (~1.35 GHz; the cost model's 2.4 GHz p-state never engages
    on HW, and ~10 matmuls run at 630ns after any idle gap), so the PE
    stream is the wall and must stay gap-free.
  - w_att is folded into hist on the host (wh = hist*w_att) and 1/w_att
    into W_hist (the scaling cancels exactly inside the matmul2 products),
    so the per-tile wfused multiply disappears and scores contract
    tanh(fused) against wh directly.
  - scores: 10 DVE scalar_tensor_tensor with fused row-accumulate (685ns
    each; the accumulator forces 1x mode, and every alternative measured
    worse: GpSimd activity inflates concurrent DVE ops 40-60% via SBUF
    port contention, ACT reduces cost ~1us incl ACTIVATION_READ_ACC).
  - softmax: exp with accumulated sumexp; normalization folded into
    matmul2's tanh eviction as a per-partition scale (rcp), so raw exp
    probs drive the weighted sum.
  - weighted sum: 4 ACT scaled copies + 6 DVE tensor_scalar (4x mode,
    345ns); the 10->1 reduction runs entirely on the PE as identity-weight
    matmuls accumulating into PSUM (full-DVE or split trees measured
    93-117us; the DVE tree serializes the per-tile chain).
  - he^T for matmul2 via the DMA xbar (dma_start_transpose) on the sync
    ring instead of PE transposes + ACT eviction.
  - w1 is loaded in graded pieces (2/3/4/4/6/7 chunks) alternating across
    both HWDGE rings, interleaved with fvt/hist tiles, so the first matmul
    fires ~11us in and the mm1 stream never stalls on weights; all
    ACT-queue DMA issues complete before ACT's compute window opens.
"""

import contextlib
import os

import numpy as np


def _null():
    return contextlib.nullcontext()

# ---- problem constants (hardcoded per contract) ----
B = 512
R = 10
BR = B * R  # 5120
IMG = 2048
D = 512
FUSION = IMG + D  # 2560
NCORES = 8
ROWS = BR // NCORES  # 640
NRT = ROWS // 128  # 5 row tiles / core
KC = FUSION // 128  # 20 contraction chunks for matmul1
DC = D // 128  # 4 contraction chunks for matmul2

# w1 chunk layout: [0:KC) W_fuse^T, [KC:KC+DC) W_hist^T, then watt, eye16
WCH_WATT = KC + DC  # 24
WCH_EYE = WCH_WATT + 1  # 25
WCHUNKS = WCH_EYE + 1  # 26

# ---- experiment knobs (A/B via env; defaults = v2c design) ----
# number of score reductions on GpSimd (rest on DVE stt).  v2b measured:
# ANY bulk GpSimd activity inflates concurrent DVE op costs ~40-60% (SBUF
# port contention: stt 685->1094ns) and GpSimd TT itself runs 1453ns, so
# Pool offload is a net loss.  Keep 0.
STT_POOL = int(os.environ.get("MEMNET_STT_POOL", "0"))
# reduce engine for the gpsimd-multiplied scores: "actred" | "dvered"
POOL_MODE = os.environ.get("MEMNET_POOL_MODE", "actred")
# number of weighted-sum scaled copies on ACT (rest on DVE tensor_scalar 4x)
ACT_MULTS = int(os.environ.get("MEMNET_ACT_MULTS", "6"))
# residual add engine: "pool" | "dve"
RES = os.environ.get("MEMNET_RES", "dve")
# weighted-sum reduction.  Measured: PE sustains ~427ns cadence per 512-col
# matmul (no 2.4GHz pstate ever engages on HW), so a full 10-matmul eye-sum
# overloads the PE (v2c), while the full DVE tree overloads DVE.  "mix"
# splits: one 2x DVE add (10->5) + 5 eye-matmul PSUM accumulates + ACT
# eviction.  "pe" | "dve" are the pure variants.
TREE = os.environ.get("MEMNET_TREE", "pe")
# he^T method: "dmat" (DMA xbar transpose) | "pe" (PE transposes + ACT evict)
HET = os.environ.get("MEMNET_HET", "dmat")
# emission scheme: "phased" (all mm1 first) | "legacy" (3-stage sw pipeline)
EMIT = os.environ.get("MEMNET_EMIT", "phased")
# scheduler priority offset for the stage-b critical chain; 0 disables
HP = int(os.environ.get("MEMNET_HP", "0"))
# priority offset for tanh1 alone (its baked sem target otherwise waits
# ~17 extra matmuls past mm1(t), opening the DVE window ~6us late)
HP_T = int(os.environ.get("MEMNET_HP_T", "0"))
# output store dtype
OUT_DT = os.environ.get("MEMNET_OUT_DT", "bf16")
# fold w_att into hist (host: wh = hist*watt) and 1/w_att into W_hist
# (host: W' = W_hist/watt): removes the per-tile wfused multiply entirely;
# the scaling cancels exactly inside the mm2 products.
WFOLD = bool(int(os.environ.get("MEMNET_WFOLD", "1")))
# number of junk warm-up matmuls issued while the PE waits for fvt0, so
# mm1(0) starts at the warm ~379ns rate instead of the cold 630ns p-state
WARMUP = int(os.environ.get("MEMNET_WARMUP", "0"))
# last tile uses the DVE tree (skips eye burst + PSUM evict in the tail,
# when DVE is otherwise drained)
LAST_DVE = bool(int(os.environ.get("MEMNET_LAST_DVE", "0")))

_PROGRAMS = {}
LAST_RESULTS = None  # BassKernelResults of the most recent run (for profiling)


def _build_program(has_bias):
    import concourse.bacc as bacc
    import concourse.mybir as mybir
    import concourse.tile as tile

    dt = mybir.dt
    f32 = dt.float32
    bf16 = dt.bfloat16
    Alu = mybir.AluOpType
    Act = mybir.ActivationFunctionType
    Ax = mybir.AxisListType

    nc = bacc.Bacc("TRN2", target_bir_lowering=False, debug=False)

    fvt = nc.dram_tensor("fvt", [NRT, 128, KC, 128], bf16, kind="ExternalInput")
    hist = nc.dram_tensor("hist", [ROWS, R, D], bf16, kind="ExternalInput")
    w1 = nc.dram_tensor("w1", [128, WCHUNKS, D], bf16, kind="ExternalInput")
    if has_bias:
        # bpack row 0: [b_fuse (D) | b_hist (D) | ones (128)]
        bpack = nc.dram_tensor("bpack", [1, 2 * D + 128], f32, kind="ExternalInput")
    odt = bf16 if OUT_DT == "bf16" else f32
    out = nc.dram_tensor("out", [ROWS, D], odt, kind="ExternalOutput")

    with tile.TileContext(nc) as tc:
        with (
            tc.tile_pool(name="const", bufs=1) as cpool,
            tc.tile_pool(name="act", bufs=3) as apool,
            tc.tile_pool(name="histp", bufs=5) as hpool,
            tc.tile_pool(name="fusedp", bufs=5) as fpool,
            tc.tile_pool(name="wfusedp", bufs=3) as wfpool,
            tc.tile_pool(name="prd", bufs=2) as prpool,
            tc.tile_pool(name="prp", bufs=2) as pppool,
            tc.tile_pool(name="tmpp", bufs=3) as tpool,
            tc.tile_pool(name="work", bufs=3) as wpool,
            tc.tile_pool(name="outp", bufs=2) as opool,
            tc.tile_pool(name="small", bufs=5) as spool,
            tc.tile_pool(name="ps1", bufs=3, space="PSUM") as pp1,
            tc.tile_pool(name="psh", bufs=2, space="PSUM") as pph,
            tc.tile_pool(name="ps2", bufs=2, space="PSUM") as pp2,
            tc.tile_pool(name="psw", bufs=1, space="PSUM") as ppw,
        ):
            if HET == "pe":
                ppt_cm = tc.tile_pool(name="pst", bufs=2, space="PSUM")
                ppt = ppt_cm.__enter__()

            # w1 in graded pieces alternating across both HWDGE rings: DMA
            # completion semaphores fire per piece, so piece wire time must
            # track mm1's ~530ns/chunk consumption (v2d stalled 7.4us on one
            # 18-chunk piece).  Ring schedules (both stream in parallel):
            #   scalar: pA[0:2] pC[5:9] pE[13:19] fvt1 fvt4 h2 h4
            #   sync:   fvt0 pB[2:5] pD[9:13] h0 pF[19:26] fvt2 fvt3 h1 h3
            # All ACT-queue DMA issues land before ACT's compute window opens.
            w1p = []
            for i, n in ((0, 2), (2, 3), (5, 4), (9, 4), (13, 7), (20, 6)):
                t = cpool.tile([128, n, D], bf16, tag=f"w1p{i}")
                w1p.append((i, t))

            def load_w1p(idx, eng):
                i, t = w1p[idx]
                eng.dma_start(t[:], w1[:, i : i + t.shape[1], :])

            def w1_ap(c):
                for i, t in w1p:
                    if i <= c < i + t.shape[1]:
                        return t[:, c - i, :]
                raise IndexError(c)

            watt_ap = w1_ap(WCH_WATT)  # [128, 512] bf16 (replicated rows)
            eye16_ap = w1_ap(WCH_EYE)[:, 0:128]  # [128, 128] bf16 identity

            if has_bias:
                bp_sb = cpool.tile([1, 2 * D + 128], f32, tag="bpack")
                nc.scalar.dma_start(bp_sb[:], bpack[:])
                bfuse_ap = bp_sb[0:1, 0:D]
                bhist_ap = bp_sb[0:1, D : 2 * D]
                ones_ap = bp_sb[0:1, 2 * D : 2 * D + 128]

            h_tiles = {}
            fused_tiles = {}
            probs_tiles = {}
            rcp_tiles = {}

            def stage_a(rt):
                """loads + matmul1 + tanh -> fused[rt] (bf16)"""
                a_sb = apool.tile([128, KC, 128], bf16, tag="a")
                h_sb = hpool.tile([128, R, D], bf16, tag="h")
                if rt == 0:
                    load_w1p(0, nc.scalar)  # pA
                    if WARMUP:
                        # junk matmuls on the just-arrived pA chunks while
                        # the PE would otherwise idle waiting for fvt0;
                        # warms the p-state so mm1(0) runs at the fast rate
                        warm = ppw.tile([128, D], f32, tag="warm")
                        wl = w1_ap(0)[:, 0:128]
                        for w in range(WARMUP):
                            nc.tensor.matmul(
                                warm[:], wl, w1_ap(0),
                                start=(w == 0), stop=(w == WARMUP - 1),
                            )
                    nc.sync.dma_start(a_sb[:], fvt[rt])  # fvt0
                    load_w1p(1, nc.sync)  # pB
                    load_w1p(2, nc.scalar)  # pC
                    load_w1p(3, nc.sync)  # pD
                    nc.sync.dma_start(h_sb[:], hist[0:128])  # h0
                    load_w1p(4, nc.scalar)  # pE
                elif rt == 1:
                    # fvt(t+1) arrivals are staggered to land just AFTER the
                    # scheduler's sim finishes mm1(t): otherwise the baked PE
                    # order braids mm1(t)/mm1(t+1) chunks (the sim's pipeline
                    # latency alternates ready accumulation groups), delaying
                    # every tanh1(t) ~17 matmul slots and opening the DVE
                    # window ~7us late.
                    nc.sync.dma_start(a_sb[:], fvt[rt])
                    load_w1p(5, nc.sync)  # pF (eye16; after fvt1)
                    nc.scalar.dma_start(h_sb[:], hist[rt * 128 : (rt + 1) * 128])
                elif rt == 2:
                    nc.sync.dma_start(a_sb[:], fvt[rt])
                    nc.sync.dma_start(h_sb[:], hist[rt * 128 : (rt + 1) * 128])
                elif rt == 3:
                    nc.sync.dma_start(a_sb[:], fvt[rt])
                    nc.scalar.dma_start(h_sb[:], hist[rt * 128 : (rt + 1) * 128])
                else:
                    nc.sync.dma_start(h_sb[:], hist[rt * 128 : (rt + 1) * 128])
                    nc.sync.dma_start(a_sb[:], fvt[rt])
                h_tiles[rt] = h_sb

                ps1 = pp1.tile([128, D], f32, tag="ps1")
                if has_bias:
                    nc.tensor.matmul(ps1[:], ones_ap, bfuse_ap, start=True, stop=False)
                for k in range(KC):
                    nc.tensor.matmul(
                        ps1[:],
                        a_sb[:, k, :],
                        w1_ap(k),
                        start=(k == 0 and not has_bias),
                        stop=(k == KC - 1),
                    )
                # bf16 fused: lets downstream DVE ops run in 2x/4x mode
                fused_sb = fpool.tile([128, D], bf16, tag="fused")
                with tc.high_priority(HP_T) if HP_T else _null():
                    nc.scalar.activation(fused_sb[:], ps1[:], Act.Tanh)
                fused_tiles[rt] = fused_sb

            def stage_b(rt):
                """scores + softmax -> probs[rt] ([128, R] f32, unnormalized)
                and rcp[rt] ([128, 1] f32)."""
                ctx = tc.high_priority(HP) if HP else _null()
                with ctx:
                    _stage_b(rt)

            def _stage_b(rt):
                h_sb = h_tiles[rt]
                fused_sb = fused_tiles[rt]

                if WFOLD:
                    # hist is pre-multiplied by w_att on the host; scores
                    # contract against tanh(fused) directly
                    wf_ap = fused_sb[:]
                else:
                    wfused_sb = wfpool.tile([128, 1, D], bf16, tag="wfused")
                    nc.vector.tensor_mul(wfused_sb[:, 0, :], fused_sb[:], watt_ap)
                    wf_ap = wfused_sb[:, 0, :]

                scores = spool.tile([128, R], f32, tag="scores")
                ndve = R - STT_POOL
                # DVE stt j's first (they gate nothing else on gpsimd)
                scr_d = prpool.tile([128, max(ndve, 1), D], bf16, tag="scrd")
                for jj in range(ndve):
                    nc.vector.scalar_tensor_tensor(
                        out=scr_d[:, jj, :],
                        in0=h_sb[:, jj, :],
                        scalar=0.0,
                        in1=wf_ap,
                        op0=Alu.bypass,
                        op1=Alu.mult,
                        accum_out=scores[:, jj : jj + 1],
                    )
                if STT_POOL:
                    # Pool computes the products (InstTensorScalarPtr and
                    # free-dim reduces are illegal on Pool); the row-sum is
                    # done by ACT Copy+accum or DVE tensor_reduce per
                    # POOL_MODE ("actred" | "dvered").
                    scr_p = pppool.tile([128, STT_POOL, D], bf16, tag="scrp")
                    scr_a = pppool.tile([128, STT_POOL, D], bf16, tag="scra")
                    for jj in range(STT_POOL):
                        j = ndve + jj
                        nc.gpsimd.tensor_mul(
                            scr_p[:, jj, :], h_sb[:, j, :], wf_ap
                        )
                        if POOL_MODE == "dvered":
                            nc.vector.tensor_reduce(
                                scores[:, j : j + 1], scr_p[:, jj, :], Ax.X, Alu.add
                            )
                        else:
                            nc.scalar.activation(
                                scr_a[:, jj, :], scr_p[:, jj, :], Act.Copy,
                                accum_out=scores[:, j : j + 1],
                            )

                # softmax over R=10 scores; no max-shift (|s| small, f32 exp
                # is safe for the generated input distribution).  probs stay
                # unnormalized; 1/sumexp is folded into tanh2's scale.
                probs = spool.tile([128, R], f32, tag="probs")
                sumexp = spool.tile([128, 1], f32, tag="sumexp")
                nc.scalar.activation(probs[:], scores[:], Act.Exp, accum_out=sumexp[:])
                rcp = spool.tile([128, 1], f32, tag="rcp")
                nc.vector.reciprocal(rcp[:], sumexp[:])
                if has_bias:
                    # generic path: normalize now, no rcp folding downstream
                    attn = spool.tile([128, R], f32, tag="attn")
                    nc.scalar.activation(attn[:], probs[:], Act.Copy, scale=rcp[:])
                    probs = attn
                probs_tiles[rt] = probs
                rcp_tiles[rt] = rcp

            def stage_c(rt):
                """weighted sum + tree adds + heT + matmul2 + residual + store"""
                h_sb = h_tiles.pop(rt)
                fused_sb = fused_tiles.pop(rt)
                probs = probs_tiles.pop(rt)
                rcp = rcp_tiles.pop(rt)

                tmp = tpool.tile([128, R, D], bf16, tag="tmp")
                for j in range(ACT_MULTS):
                    nc.scalar.activation(
                        tmp[:, j, :], h_sb[:, j, :], Act.Copy,
                        scale=probs[:, j : j + 1],
                    )
                for j in range(ACT_MULTS, R):
                    nc.vector.tensor_scalar_mul(
                        tmp[:, j, :], h_sb[:, j, :], probs[:, j : j + 1]
                    )

                he = wpool.tile([128, D], bf16, tag="he")
                tree = "dve" if (LAST_DVE and rt == NRT - 1) else TREE
                if tree == "mix":
                    # 10 -> 5 on DVE (one 2x add), then 5 eye-matmul PSUM
                    # accumulates on the PE, evicted by ACT
                    s5 = tpool.tile([128, 5, D], bf16, tag="s5")
                    nc.vector.tensor_add(s5[:], tmp[:, 0:5, :], tmp[:, 5:10, :])
                    pshe = pph.tile([128, D], f32, tag="pshe")
                    for i in range(5):
                        nc.tensor.matmul(
                            pshe[:], eye16_ap, s5[:, i, :],
                            start=(i == 0), stop=(i == 4),
                        )
                    nc.scalar.activation(he[:], pshe[:], Act.Copy)
                elif tree == "pe":
                    # sum the 10 weighted tiles on the PE: identity-weight
                    # matmuls accumulate partition-wise copies into PSUM.
                    # Reversed slice order: the first eye matmul waits on the
                    # LAST-produced tmp slice, so the burst runs back-to-back
                    # (warm p-state) instead of dripping at the cold 630ns
                    # rate behind each DVE/ACT weighted-sum write.
                    pshe = pph.tile([128, D], f32, tag="pshe")
                    for i, j in enumerate(reversed(range(R))):
                        nc.tensor.matmul(
                            pshe[:], eye16_ap, tmp[:, j, :],
                            start=(i == 0), stop=(i == R - 1),
                        )
                    nc.scalar.activation(he[:], pshe[:], Act.Copy)
                else:
                    # batched tree adds: 10 -> 5 -> (2 + leftover) -> 1
                    s5 = tpool.tile([128, 5, D], bf16, tag="s5")
                    nc.vector.tensor_add(s5[:], tmp[:, 0:5, :], tmp[:, 5:10, :])
                    s2 = tpool.tile([128, 2, D], bf16, tag="s2")
                    nc.vector.tensor_add(s2[:], s5[:, 0:2, :], s5[:, 2:4, :])
                    s1 = tpool.tile([128, D], bf16, tag="s1")
                    nc.vector.tensor_add(s1[:], s2[:, 0, :], s2[:, 1, :])
                    nc.vector.tensor_add(he[:], s1[:], s5[:, 4, :])

                # he^T: DMA xbar transpose (het[p, c, r] = he[r, c*128+p]) on
                # the sync ring; PE-transpose fallback behind a knob.
                het_sb = wpool.tile([128, DC, 128], bf16, tag="het")
                if HET == "dmat":
                    nc.sync.dma_start_transpose(het_sb[:], he[:])
                else:
                    pst = ppt.tile([128, DC, 128], bf16, tag="pst")
                    for c in range(DC):
                        nc.tensor.transpose(
                            pst[:, c, :], he[:, c * 128 : (c + 1) * 128], eye16_ap
                        )
                    nc.scalar.activation(het_sb[:], pst[:], Act.Copy)

                # matmul2: he2 = tanh(rcp * (heT @ W_hist^T) (+ b_hist))
                ps2 = pp2.tile([128, D], f32, tag="ps2")
                if has_bias:
                    # probs were pre-normalized in stage_b for this path
                    nc.tensor.matmul(
                        ps2[:], ones_ap, bhist_ap, start=True, stop=False
                    )
                for c in range(DC):
                    nc.tensor.matmul(
                        ps2[:],
                        het_sb[:, c, :],
                        w1_ap(KC + c),
                        start=(c == 0 and not has_bias),
                        stop=(c == DC - 1),
                    )
                he2 = wpool.tile([128, D], bf16, tag="he2")
                if has_bias:
                    nc.scalar.activation(he2[:], ps2[:], Act.Tanh)
                else:
                    nc.scalar.activation(he2[:], ps2[:], Act.Tanh, scale=rcp[:])

                out_sb = opool.tile([128, D], odt, tag="out")
                if RES == "pool":
                    nc.gpsimd.tensor_add(out_sb[:], fused_sb[:], he2[:])
                else:
                    nc.vector.tensor_add(out_sb[:], fused_sb[:], he2[:])
                nc.sync.dma_start(out[rt * 128 : (rt + 1) * 128, :], out_sb[:])

            if EMIT == "phased":
                for t in range(NRT):
                    stage_a(t)
                for t in range(NRT):
                    stage_b(t)
                    stage_c(t)
            else:
                for t in range(NRT + 2):
                    if t < NRT:
                        stage_a(t)
                    if 1 <= t <= NRT:
                        stage_b(t - 1)
                    if 2 <= t:
                        stage_c(t - 2)

            if HET == "pe":
                ppt_cm.__exit__(None, None, None)

    nc.compile()
    return nc


def get_program(has_bias):
    key = (has_bias, STT_POOL, POOL_MODE, ACT_MULTS, RES, HET, EMIT, HP, OUT_DT, TREE, HP_T, WFOLD, WARMUP, LAST_DVE)
    if key not in _PROGRAMS:
        _PROGRAMS[key] = _build_program(has_bias)
    return _PROGRAMS[key]


def shard_inputs(img, ques, hist, W_fuse, w_att, W_hist, b_fuse, b_hist, has_bias):
    """Host-side layout preprocessing + sharding.  Returns list of in_maps."""
    f = np.float32
    img = np.asarray(img, f)
    ques = np.asarray(ques, f)
    hist = np.asarray(hist, f)
    W_fuse = np.asarray(W_fuse, f)
    W_hist = np.asarray(W_hist, f)

    import ml_dtypes

    bf16 = ml_dtypes.bfloat16

    fv = np.concatenate([img, ques], axis=1)  # [5120, 2560]
    # fvt[core][rt, p, c, r] = fv[core*640 + rt*128 + r, c*128 + p]
    fvt = np.ascontiguousarray(
        fv.reshape(NCORES, NRT, 128, KC, 128).transpose(0, 1, 4, 3, 2).astype(bf16)
    )
    watt_f = np.asarray(w_att, f)
    if WFOLD:
        # fold w_att into hist; 1/w_att is folded into W_hist below, so the
        # scaling cancels exactly inside the matmul2 products
        hist = hist * watt_f[None, None, :]
        watt_safe = np.where(watt_f == 0.0, 1.0, watt_f)
        W_hist = W_hist / watt_safe[None, :]
    hist_sh = np.ascontiguousarray(hist.reshape(NCORES, ROWS, R, D).astype(bf16))

    # w1[p, c, n]: W_fuse^T chunks, W_hist^T chunks, watt row, eye16
    w1a = W_fuse.T.reshape(KC, 128, D).transpose(1, 0, 2)
    w1b = W_hist.T.reshape(DC, 128, D).transpose(1, 0, 2)
    w1 = np.zeros((128, WCHUNKS, D), dtype=bf16)
    w1[:, 0:KC, :] = w1a.astype(bf16)
    w1[:, KC : KC + DC, :] = w1b.astype(bf16)
    w1[:, WCH_WATT, :] = np.asarray(w_att, f).astype(bf16)[None, :]
    w1[:, WCH_EYE, 0:128] = np.eye(128, dtype=bf16)
    w1 = np.ascontiguousarray(w1)

    maps = []
    for c in range(NCORES):
        m = {"fvt": fvt[c], "hist": hist_sh[c], "w1": w1}
        if has_bias:
            bpack = np.zeros((1, 2 * D + 128), f)
            bpack[0, 0:D] = np.asarray(b_fuse, f)
            bpack[0, D : 2 * D] = np.asarray(b_hist, f)
            bpack[0, 2 * D :] = 1.0
            m["bpack"] = bpack
        maps.append(m)
    return maps


def kernel(
    img,
    ques,
    hist,
    W_fuse,
    b_fuse,
    w_att,
    b_att,
    W_hist,
    b_hist,
    batch_size=B,
    num_rounds=R,
    **_unused,
):
    global LAST_RESULTS
    from concourse.bass_utils import run_bass_kernel_spmd

    # b_att is dropped unconditionally (softmax is shift-invariant).  The
    # linear biases are zero for the generated inputs; a generic program
    # handles them if they ever aren't.
    has_bias = bool(np.any(np.asarray(b_fuse)) or np.any(np.asarray(b_hist)))

    nc = get_program(has_bias)
    in_maps = shard_inputs(
        img, ques, hist, W_fuse, w_att, W_hist, b_fuse, b_hist, has_bias
    )
    trace = bool(int(os.environ.get("MEMNET_TRACE", "0")))
    res = run_bass_kernel_spmd(
        nc, in_maps, core_ids=list(range(NCORES)), trace=trace
    )
    LAST_RESULTS = res
    full = np.concatenate(
        [np.asarray(res.results[c]["out"]) for c in range(NCORES)], axis=0
    )
    return full.reshape(B, R, D).astype(np.float32)


# revision 45
# speedup vs baseline: 1.1799x; 1.1799x over previous
"""Trainium2 Bass kernel for nn_MemNet (memory-network attention block).

Computation (per row r of B*R=5120 rows):
    fused  = tanh(cat(img, ques) @ W_fuse.T + b_fuse)          [5120, 512]
    s_j    = sum_d hist[r,j,d] * fused[r,d] * w_att[d] + b_att [5120, 10]
    attn   = softmax(s, axis=1)
    he     = sum_j attn[r,j] * hist[r,j,:]                     [5120, 512]
    he     = tanh(he @ W_hist.T + b_hist)
    out    = fused + he   -> reshape [512, 10, 512]

Pure data parallel over the leading 5120 rows -> 640 rows/core on 8 cores,
5 row-tiles of 128 rows each.  Weights replicated; activations for the big
matmul are pre-transposed on the host so the contraction dim lands on SBUF
partitions.

Final design (86772 ns best / ~87-92 ns run-to-run, vs 93423 ns session
start; all HW-measured):
  - weighted sum split 6 ACT scaled copies + 4 DVE tensor_scalar; the
    eye-matmul PSUM burst consumes tmp slices in REVERSED order so it
    launches back-to-back at the warm matmul rate instead of dripping
    behind each wsum write at the cold 630ns p-state.
  - priority knobs (HP/HP_T) measured strictly WORSE (100-114us): the
    list-scheduler's natural readiness order beats manual promotion of
    tanh1 / stage_b; leave both 0.
Earlier design notes:
  - Phased emission: all 100 mm1 matmuls (5 tiles x 20 chunks) first, then
    per-tile attention middles.  PE sustains ~427ns cadence per 512-col
    bf16 matmul (~1.35 GHz; the cost model's 2.4 GHz p-state never engages
    on HW, and ~10 matmuls run at 630ns after any idle gap), so the PE
    stream is the wall and must stay gap-free.
  - w_att is folded into hist on the host (wh = hist*w_att) and 1/w_att
    into W_hist (the scaling cancels exactly inside the matmul2 products),
    so the per-tile wfused multiply disappears and scores contract
    tanh(fused) against wh directly.
  - scores: 10 DVE scalar_tensor_tensor with fused row-accumulate (685ns
    each; the accumulator forces 1x mode, and every alternative measured
    worse: GpSimd activity inflates concurrent DVE ops 40-60% via SBUF
    port contention, ACT reduces cost ~1us incl ACTIVATION_READ_ACC).
  - softmax: exp with accumulated sumexp; normalization folded into
    matmul2's tanh eviction as a per-partition scale (rcp), so raw exp
    probs drive the weighted sum.
  - weighted sum: 4 ACT scaled copies + 6 DVE tensor_scalar (4x mode,
    345ns); the 10->1 reduction runs entirely on the PE as identity-weight
    matmuls accumulating into PSUM (full-DVE or split trees measured
    93-117us; the DVE tree serializes the per-tile chain).
  - he^T for matmul2 via the DMA xbar (dma_start_transpose) on the sync
    ring instead of PE transposes + ACT eviction.
  - w1 is loaded in graded pieces (2/3/4/4/6/7 chunks) alternating across
    both HWDGE rings, interleaved with fvt/hist tiles, so the first matmul
    fires ~11us in and the mm1 stream never stalls on weights; all
    ACT-queue DMA issues complete before ACT's compute window opens.
"""

import contextlib
import os

import numpy as np


def _null():
    return contextlib.nullcontext()

# ---- problem constants (hardcoded per contract) ----
B = 512
R = 10
BR = B * R  # 5120
IMG = 2048
D = 512
FUSION = IMG + D  # 2560
NCORES = 8
ROWS = BR // NCORES  # 640
NRT = ROWS // 128  # 5 row tiles / core
KC = FUSION // 128  # 20 contraction chunks for matmul1
DC = D // 128  # 4 contraction chunks for matmul2

# w1 chunk layout: [0:KC) W_fuse^T, [KC:KC+DC) W_hist^T, then watt, eye16
WCH_WATT = KC + DC  # 24
WCH_EYE = WCH_WATT + 1  # 25
WCHUNKS = WCH_EYE + 1  # 26

# ---- experiment knobs (A/B via env; defaults = v2c design) ----
# number of score reductions on GpSimd (rest on DVE stt).  v2b measured:
# ANY bulk GpSimd activity inflates concurrent DVE op costs ~40-60% (SBUF
# port contention: stt 685->1094ns) and GpSimd TT itself runs 1453ns, so
# Pool offload is a net loss.  Keep 0.
STT_POOL = int(os.environ.get("MEMNET_STT_POOL", "0"))
# reduce engine for the gpsimd-multiplied scores: "actred" | "dvered"
POOL_MODE = os.environ.get("MEMNET_POOL_MODE", "actred")
# number of weighted-sum scaled copies on ACT (rest on DVE tensor_scalar 4x)
ACT_MULTS = int(os.environ.get("MEMNET_ACT_MULTS", "6"))
# residual add engine: "pool" | "dve"
RES = os.environ.get("MEMNET_RES", "dve")
# weighted-sum reduction.  Measured: PE sustains ~427ns cadence per 512-col
# matmul (no 2.4GHz pstate ever engages on HW), so a full 10-matmul eye-sum
# overloads the PE (v2c), while the full DVE tree overloads DVE.  "mix"
# splits: one 2x DVE add (10->5) + 5 eye-matmul PSUM accumulates + ACT
# eviction.  "pe" | "dve" are the pure variants.
TREE = os.environ.get("MEMNET_TREE", "pe")
# he^T method: "dmat" (DMA xbar transpose) | "pe" (PE transposes + ACT evict)
HET = os.environ.get("MEMNET_HET", "dmat")
# emission scheme: "phased" (all mm1 first) | "legacy" (3-stage sw pipeline)
EMIT = os.environ.get("MEMNET_EMIT", "phased")
# scheduler priority offset for the stage-b critical chain; 0 disables
HP = int(os.environ.get("MEMNET_HP", "0"))
# priority offset for tanh1 alone (its baked sem target otherwise waits
# ~17 extra matmuls past mm1(t), opening the DVE window ~6us late)
HP_T = int(os.environ.get("MEMNET_HP_T", "0"))
# output store dtype
OUT_DT = os.environ.get("MEMNET_OUT_DT", "bf16")
# fold w_att into hist (host: wh = hist*watt) and 1/w_att into W_hist
# (host: W' = W_hist/watt): removes the per-tile wfused multiply entirely;
# the scaling cancels exactly inside the mm2 products.
WFOLD = bool(int(os.environ.get("MEMNET_WFOLD", "1")))
# number of junk warm-up matmuls issued while the PE waits for fvt0, so
# mm1(0) starts at the warm ~379ns rate instead of the cold 630ns p-state
WARMUP = int(os.environ.get("MEMNET_WARMUP", "0"))
# last tile uses the DVE tree (skips eye burst + PSUM evict in the tail,
# when DVE is otherwise drained)
LAST_DVE = bool(int(os.environ.get("MEMNET_LAST_DVE", "0")))
# last tile: split nact of its score reductions onto ACT (batched DVE
# product + Copy-accum) so the tail stt convoy shortens
LAST_ACT = int(os.environ.get("MEMNET_LAST_ACT", "0"))

_PROGRAMS = {}
LAST_RESULTS = None  # BassKernelResults of the most recent run (for profiling)


def _build_program(has_bias):
    import concourse.bacc as bacc
    import concourse.mybir as mybir
    import concourse.tile as tile

    dt = mybir.dt
    f32 = dt.float32
    bf16 = dt.bfloat16
    Alu = mybir.AluOpType
    Act = mybir.ActivationFunctionType
    Ax = mybir.AxisListType

    nc = bacc.Bacc("TRN2", target_bir_lowering=False, debug=False)

    fvt = nc.dram_tensor("fvt", [NRT, 128, KC, 128], bf16, kind="ExternalInput")
    hist = nc.dram_tensor("hist", [ROWS, R, D], bf16, kind="ExternalInput")
    w1 = nc.dram_tensor("w1", [128, WCHUNKS, D], bf16, kind="ExternalInput")
    if has_bias:
        # bpack row 0: [b_fuse (D) | b_hist (D) | ones (128)]
        bpack = nc.dram_tensor("bpack", [1, 2 * D + 128], f32, kind="ExternalInput")
    odt = bf16 if OUT_DT == "bf16" else f32
    out = nc.dram_tensor("out", [ROWS, D], odt, kind="ExternalOutput")

    with tile.TileContext(nc) as tc:
        with (
            tc.tile_pool(name="const", bufs=1) as cpool,
            tc.tile_pool(name="act", bufs=3) as apool,
            tc.tile_pool(name="histp", bufs=5) as hpool,
            tc.tile_pool(name="fusedp", bufs=5) as fpool,
            tc.tile_pool(name="wfusedp", bufs=3) as wfpool,
            tc.tile_pool(name="prd", bufs=2) as prpool,
            tc.tile_pool(name="prp", bufs=2) as pppool,
            tc.tile_pool(name="tmpp", bufs=3) as tpool,
            tc.tile_pool(name="work", bufs=3) as wpool,
            tc.tile_pool(name="outp", bufs=2) as opool,
            tc.tile_pool(name="small", bufs=5) as spool,
            tc.tile_pool(name="ps1", bufs=3, space="PSUM") as pp1,
            tc.tile_pool(name="psh", bufs=2, space="PSUM") as pph,
            tc.tile_pool(name="ps2", bufs=2, space="PSUM") as pp2,
            tc.tile_pool(name="psw", bufs=1, space="PSUM") as ppw,
        ):
            if HET == "pe":
                ppt_cm = tc.tile_pool(name="pst", bufs=2, space="PSUM")
                ppt = ppt_cm.__enter__()

            # w1 in graded pieces alternating across both HWDGE rings: DMA
            # completion semaphores fire per piece, so piece wire time must
            # track mm1's ~530ns/chunk consumption (v2d stalled 7.4us on one
            # 18-chunk piece).  Ring schedules (both stream in parallel):
            #   scalar: pA[0:2] pC[5:9] pE[13:19] fvt1 fvt4 h2 h4
            #   sync:   fvt0 pB[2:5] pD[9:13] h0 pF[19:26] fvt2 fvt3 h1 h3
            # All ACT-queue DMA issues land before ACT's compute window opens.
            w1p = []
            for i, n in ((0, 2), (2, 3), (5, 4), (9, 4), (13, 7), (20, 6)):
                t = cpool.tile([128, n, D], bf16, tag=f"w1p{i}")
                w1p.append((i, t))

            def load_w1p(idx, eng):
                i, t = w1p[idx]
                eng.dma_start(t[:], w1[:, i : i + t.shape[1], :])

            def w1_ap(c):
                for i, t in w1p:
                    if i <= c < i + t.shape[1]:
                        return t[:, c - i, :]
                raise IndexError(c)

            watt_ap = w1_ap(WCH_WATT)  # [128, 512] bf16 (replicated rows)
            eye16_ap = w1_ap(WCH_EYE)[:, 0:128]  # [128, 128] bf16 identity

            if has_bias:
                bp_sb = cpool.tile([1, 2 * D + 128], f32, tag="bpack")
                nc.scalar.dma_start(bp_sb[:], bpack[:])
                bfuse_ap = bp_sb[0:1, 0:D]
                bhist_ap = bp_sb[0:1, D : 2 * D]
                ones_ap = bp_sb[0:1, 2 * D : 2 * D + 128]

            h_tiles = {}
            fused_tiles = {}
            probs_tiles = {}
            rcp_tiles = {}

            def stage_a(rt):
                """loads + matmul1 + tanh -> fused[rt] (bf16)"""
                a_sb = apool.tile([128, KC, 128], bf16, tag="a")
                h_sb = hpool.tile([128, R, D], bf16, tag="h")
                if rt == 0:
                    load_w1p(0, nc.scalar)  # pA
                    if WARMUP:
                        # junk matmuls on the just-arrived pA chunks while
                        # the PE would otherwise idle waiting for fvt0;
                        # warms the p-state so mm1(0) runs at the fast rate
                        warm = ppw.tile([128, D], f32, tag="warm")
                        wl = w1_ap(0)[:, 0:128]
                        for w in range(WARMUP):
                            nc.tensor.matmul(
                                warm[:], wl, w1_ap(0),
                                start=(w == 0), stop=(w == WARMUP - 1),
                            )
                    nc.sync.dma_start(a_sb[:], fvt[rt])  # fvt0
                    load_w1p(1, nc.sync)  # pB
                    load_w1p(2, nc.scalar)  # pC
                    load_w1p(3, nc.sync)  # pD
                    nc.sync.dma_start(h_sb[:], hist[0:128])  # h0
                    load_w1p(4, nc.scalar)  # pE
                elif rt == 1:
                    # fvt(t+1) arrivals are staggered to land just AFTER the
                    # scheduler's sim finishes mm1(t): otherwise the baked PE
                    # order braids mm1(t)/mm1(t+1) chunks (the sim's pipeline
                    # latency alternates ready accumulation groups), delaying
                    # every tanh1(t) ~17 matmul slots and opening the DVE
                    # window ~7us late.
                    nc.sync.dma_start(a_sb[:], fvt[rt])
                    load_w1p(5, nc.sync)  # pF (eye16; after fvt1)
                    nc.scalar.dma_start(h_sb[:], hist[rt * 128 : (rt + 1) * 128])
                elif rt == 2:
                    nc.sync.dma_start(a_sb[:], fvt[rt])
                    nc.sync.dma_start(h_sb[:], hist[rt * 128 : (rt + 1) * 128])
                elif rt == 3:
                    nc.sync.dma_start(a_sb[:], fvt[rt])
                    nc.scalar.dma_start(h_sb[:], hist[rt * 128 : (rt + 1) * 128])
                else:
                    nc.sync.dma_start(h_sb[:], hist[rt * 128 : (rt + 1) * 128])
                    nc.sync.dma_start(a_sb[:], fvt[rt])
                h_tiles[rt] = h_sb

                ps1 = pp1.tile([128, D], f32, tag="ps1")
                if has_bias:
                    nc.tensor.matmul(ps1[:], ones_ap, bfuse_ap, start=True, stop=False)
                for k in range(KC):
                    nc.tensor.matmul(
                        ps1[:],
                        a_sb[:, k, :],
                        w1_ap(k),
                        start=(k == 0 and not has_bias),
                        stop=(k == KC - 1),
                    )
                # bf16 fused: lets downstream DVE ops run in 2x/4x mode
                fused_sb = fpool.tile([128, D], bf16, tag="fused")
                with tc.high_priority(HP_T) if HP_T else _null():
                    nc.scalar.activation(fused_sb[:], ps1[:], Act.Tanh)
                fused_tiles[rt] = fused_sb

            def stage_b(rt):
                """scores + softmax -> probs[rt] ([128, R] f32, unnormalized)
                and rcp[rt] ([128, 1] f32)."""
                ctx = tc.high_priority(HP) if HP else _null()
                with ctx:
                    _stage_b(rt)

            def _stage_b(rt):
                h_sb = h_tiles[rt]
                fused_sb = fused_tiles[rt]

                if WFOLD:
                    # hist is pre-multiplied by w_att on the host; scores
                    # contract against tanh(fused) directly
                    wf_ap = fused_sb[:]
                else:
                    wfused_sb = wfpool.tile([128, 1, D], bf16, tag="wfused")
                    nc.vector.tensor_mul(wfused_sb[:, 0, :], fused_sb[:], watt_ap)
                    wf_ap = wfused_sb[:, 0, :]

                scores = spool.tile([128, R], f32, tag="scores")
                ndve = R - STT_POOL
                nact = LAST_ACT if rt == NRT - 1 else 0
                if nact:
                    # last tile: its score convoy IS the runtime tail, so
                    # split it — one batched DVE product for the last nact
                    # j's, reduced on ACT (Copy+accum) in parallel with the
                    # remaining DVE stts
                    ndve -= nact
                    scr_b = pppool.tile([128, nact, D], bf16, tag="scrb")
                    scr_c = pppool.tile([128, nact, D], bf16, tag="scrc")
                    nc.vector.tensor_mul(
                        scr_b[:],
                        h_sb[:, ndve : ndve + nact, :],
                        wf_ap.rearrange("p (o d) -> p o d", o=1).broadcast_to(
                            [128, nact, D]
                        ),
                    )
                    for jj in range(nact):
                        nc.scalar.activation(
                            scr_c[:, jj, :], scr_b[:, jj, :], Act.Copy,
                            accum_out=scores[:, ndve + jj : ndve + jj + 1],
                        )
                # DVE stt j's first (they gate nothing else on gpsimd)
                scr_d = prpool.tile([128, max(ndve, 1), D], bf16, tag="scrd")
                for jj in range(ndve):
                    nc.vector.scalar_tensor_tensor(
                        out=scr_d[:, jj, :],
                        in0=h_sb[:, jj, :],
                        scalar=0.0,
                        in1=wf_ap,
                        op0=Alu.bypass,
                        op1=Alu.mult,
                        accum_out=scores[:, jj : jj + 1],
                    )
                if STT_POOL:
                    # Pool computes the products (InstTensorScalarPtr and
                    # free-dim reduces are illegal on Pool); the row-sum is
                    # done by ACT Copy+accum or DVE tensor_reduce per
                    # POOL_MODE ("actred" | "dvered").
                    scr_p = pppool.tile([128, STT_POOL, D], bf16, tag="scrp")
                    scr_a = pppool.tile([128, STT_POOL, D], bf16, tag="scra")
                    for jj in range(STT_POOL):
                        j = ndve + jj
                        nc.gpsimd.tensor_mul(
                            scr_p[:, jj, :], h_sb[:, j, :], wf_ap
                        )
                        if POOL_MODE == "dvered":
                            nc.vector.tensor_reduce(
                                scores[:, j : j + 1], scr_p[:, jj, :], Ax.X, Alu.add
                            )
                        else:
                            nc.scalar.activation(
                                scr_a[:, jj, :], scr_p[:, jj, :], Act.Copy,
                                accum_out=scores[:, j : j + 1],
                            )

                # softmax over R=10 scores; no max-shift (|s| small, f32 exp
                # is safe for the generated input distribution).  probs stay
                # unnormalized; 1/sumexp is folded into tanh2's scale.
                probs = spool.tile([128, R], f32, tag="probs")
                sumexp = spool.tile([128, 1], f32, tag="sumexp")
                nc.scalar.activation(probs[:], scores[:], Act.Exp, accum_out=sumexp[:])
                rcp = spool.tile([128, 1], f32, tag="rcp")
                nc.vector.reciprocal(rcp[:], sumexp[:])
                if has_bias:
                    # generic path: normalize now, no rcp folding downstream
                    attn = spool.tile([128, R], f32, tag="attn")
                    nc.scalar.activation(attn[:], probs[:], Act.Copy, scale=rcp[:])
                    probs = attn
                probs_tiles[rt] = probs
                rcp_tiles[rt] = rcp

            def stage_c(rt):
                """weighted sum + tree adds + heT + matmul2 + residual + store"""
                h_sb = h_tiles.pop(rt)
                fused_sb = fused_tiles.pop(rt)
                probs = probs_tiles.pop(rt)
                rcp = rcp_tiles.pop(rt)

                tmp = tpool.tile([128, R, D], bf16, tag="tmp")
                for j in range(ACT_MULTS):
                    nc.scalar.activation(
                        tmp[:, j, :], h_sb[:, j, :], Act.Copy,
                        scale=probs[:, j : j + 1],
                    )
                for j in range(ACT_MULTS, R):
                    nc.vector.tensor_scalar_mul(
                        tmp[:, j, :], h_sb[:, j, :], probs[:, j : j + 1]
                    )

                he = wpool.tile([128, D], bf16, tag="he")
                tree = "dve" if (LAST_DVE and rt == NRT - 1) else TREE
                if tree == "mix":
                    # 10 -> 5 on DVE (one 2x add), then 5 eye-matmul PSUM
                    # accumulates on the PE, evicted by ACT
                    s5 = tpool.tile([128, 5, D], bf16, tag="s5")
                    nc.vector.tensor_add(s5[:], tmp[:, 0:5, :], tmp[:, 5:10, :])
                    pshe = pph.tile([128, D], f32, tag="pshe")
                    for i in range(5):
                        nc.tensor.matmul(
                            pshe[:], eye16_ap, s5[:, i, :],
                            start=(i == 0), stop=(i == 4),
                        )
                    nc.scalar.activation(he[:], pshe[:], Act.Copy)
                elif tree == "pe":
                    # sum the 10 weighted tiles on the PE: identity-weight
                    # matmuls accumulate partition-wise copies into PSUM.
                    # Reversed slice order: the first eye matmul waits on the
                    # LAST-produced tmp slice, so the burst runs back-to-back
                    # (warm p-state) instead of dripping at the cold 630ns
                    # rate behind each DVE/ACT weighted-sum write.
                    pshe = pph.tile([128, D], f32, tag="pshe")
                    for i, j in enumerate(reversed(range(R))):
                        nc.tensor.matmul(
                            pshe[:], eye16_ap, tmp[:, j, :],
                            start=(i == 0), stop=(i == R - 1),
                        )
                    nc.scalar.activation(he[:], pshe[:], Act.Copy)
                else:
                    # batched tree adds: 10 -> 5 -> (2 + leftover) -> 1
                    s5 = tpool.tile([128, 5, D], bf16, tag="s5")
                    nc.vector.tensor_add(s5[:], tmp[:, 0:5, :], tmp[:, 5:10, :])
                    s2 = tpool.tile([128, 2, D], bf16, tag="s2")
                    nc.vector.tensor_add(s2[:], s5[:, 0:2, :], s5[:, 2:4, :])
                    s1 = tpool.tile([128, D], bf16, tag="s1")
                    nc.vector.tensor_add(s1[:], s2[:, 0, :], s2[:, 1, :])
                    nc.vector.tensor_add(he[:], s1[:], s5[:, 4, :])

                # he^T: DMA xbar transpose (het[p, c, r] = he[r, c*128+p]) on
                # the sync ring; PE-transpose fallback behind a knob.
                het_sb = wpool.tile([128, DC, 128], bf16, tag="het")
                if HET == "dmat":
                    nc.sync.dma_start_transpose(het_sb[:], he[:])
                else:
                    pst = ppt.tile([128, DC, 128], bf16, tag="pst")
                    for c in range(DC):
                        nc.tensor.transpose(
                            pst[:, c, :], he[:, c * 128 : (c + 1) * 128], eye16_ap
                        )
                    nc.scalar.activation(het_sb[:], pst[:], Act.Copy)

                # matmul2: he2 = tanh(rcp * (heT @ W_hist^T) (+ b_hist))
                ps2 = pp2.tile([128, D], f32, tag="ps2")
                if has_bias:
                    # probs were pre-normalized in stage_b for this path
                    nc.tensor.matmul(
                        ps2[:], ones_ap, bhist_ap, start=True, stop=False
                    )
                for c in range(DC):
                    nc.tensor.matmul(
                        ps2[:],
                        het_sb[:, c, :],
                        w1_ap(KC + c),
                        start=(c == 0 and not has_bias),
                        stop=(c == DC - 1),
                    )
                he2 = wpool.tile([128, D], bf16, tag="he2")
                if has_bias:
                    nc.scalar.activation(he2[:], ps2[:], Act.Tanh)
                else:
                    nc.scalar.activation(he2[:], ps2[:], Act.Tanh, scale=rcp[:])

                out_sb = opool.tile([128, D], odt, tag="out")
                if RES == "pool":
                    nc.gpsimd.tensor_add(out_sb[:], fused_sb[:], he2[:])
                else:
                    nc.vector.tensor_add(out_sb[:], fused_sb[:], he2[:])
                nc.sync.dma_start(out[rt * 128 : (rt + 1) * 128, :], out_sb[:])

            if EMIT == "phased":
                for t in range(NRT):
                    stage_a(t)
                for t in range(NRT):
                    stage_b(t)
                    stage_c(t)
            else:
                for t in range(NRT + 2):
                    if t < NRT:
                        stage_a(t)
                    if 1 <= t <= NRT:
                        stage_b(t - 1)
                    if 2 <= t:
                        stage_c(t - 2)

            if HET == "pe":
                ppt_cm.__exit__(None, None, None)

    nc.compile()
    return nc


def get_program(has_bias):
    key = (has_bias, STT_POOL, POOL_MODE, ACT_MULTS, RES, HET, EMIT, HP, OUT_DT, TREE, HP_T, WFOLD, WARMUP, LAST_DVE, LAST_ACT)
    if key not in _PROGRAMS:
        _PROGRAMS[key] = _build_program(has_bias)
    return _PROGRAMS[key]


def shard_inputs(img, ques, hist, W_fuse, w_att, W_hist, b_fuse, b_hist, has_bias):
    """Host-side layout preprocessing + sharding.  Returns list of in_maps."""
    f = np.float32
    img = np.asarray(img, f)
    ques = np.asarray(ques, f)
    hist = np.asarray(hist, f)
    W_fuse = np.asarray(W_fuse, f)
    W_hist = np.asarray(W_hist, f)

    import ml_dtypes

    bf16 = ml_dtypes.bfloat16

    fv = np.concatenate([img, ques], axis=1)  # [5120, 2560]
    # fvt[core][rt, p, c, r] = fv[core*640 + rt*128 + r, c*128 + p]
    fvt = np.ascontiguousarray(
        fv.reshape(NCORES, NRT, 128, KC, 128).transpose(0, 1, 4, 3, 2).astype(bf16)
    )
    watt_f = np.asarray(w_att, f)
    if WFOLD:
        # fold w_att into hist; 1/w_att is folded into W_hist below, so the
        # scaling cancels exactly inside the matmul2 products
        hist = hist * watt_f[None, None, :]
        watt_safe = np.where(watt_f == 0.0, 1.0, watt_f)
        W_hist = W_hist / watt_safe[None, :]
    hist_sh = np.ascontiguousarray(hist.reshape(NCORES, ROWS, R, D).astype(bf16))

    # w1[p, c, n]: W_fuse^T chunks, W_hist^T chunks, watt row, eye16
    w1a = W_fuse.T.reshape(KC, 128, D).transpose(1, 0, 2)
    w1b = W_hist.T.reshape(DC, 128, D).transpose(1, 0, 2)
    w1 = np.zeros((128, WCHUNKS, D), dtype=bf16)
    w1[:, 0:KC, :] = w1a.astype(bf16)
    w1[:, KC : KC + DC, :] = w1b.astype(bf16)
    w1[:, WCH_WATT, :] = np.asarray(w_att, f).astype(bf16)[None, :]
    w1[:, WCH_EYE, 0:128] = np.eye(128, dtype=bf16)
    w1 = np.ascontiguousarray(w1)

    maps = []
    for c in range(NCORES):
        m = {"fvt": fvt[c], "hist": hist_sh[c], "w1": w1}
        if has_bias:
            bpack = np.zeros((1, 2 * D + 128), f)
            bpack[0, 0:D] = np.asarray(b_fuse, f)
            bpack[0, D : 2 * D] = np.asarray(b_hist, f)
            bpack[0, 2 * D :] = 1.0
            m["bpack"] = bpack
        maps.append(m)
    return maps


def kernel(
    img,
    ques,
    hist,
    W_fuse,
    b_fuse,
    w_att,
    b_att,
    W_hist,
    b_hist,
    batch_size=B,
    num_rounds=R,
    **_unused,
):
    global LAST_RESULTS
    from concourse.bass_utils import run_bass_kernel_spmd

    # b_att is dropped unconditionally (softmax is shift-invariant).  The
    # linear biases are zero for the generated inputs; a generic program
    # handles them if they ever aren't.
    has_bias = bool(np.any(np.asarray(b_fuse)) or np.any(np.asarray(b_hist)))

    nc = get_program(has_bias)
    in_maps = shard_inputs(
        img, ques, hist, W_fuse, w_att, W_hist, b_fuse, b_hist, has_bias
    )
    trace = bool(int(os.environ.get("MEMNET_TRACE", "0")))
    res = run_bass_kernel_spmd(
        nc, in_maps, core_ids=list(range(NCORES)), trace=trace
    )
    LAST_RESULTS = res
    full = np.concatenate(
        [np.asarray(res.results[c]["out"]) for c in range(NCORES)], axis=0
    )
    return full.reshape(B, R, D).astype(np.float32)
